# revision 14
# baseline (speedup 1.0000x reference)
"""Trainium2 Bass kernel for nn_BaselineAttention_36172214567310.

Reference computation (note the einsum 'bhqk,bhkd->bhkd' bug: the "attention
output" is v scaled by the column-sums of the softmax matrix):

    qkv = x @ w_qkv                       # [b, s, 3*H*D]
    q, k, v = split(qkv)                  # per head
    P = softmax(q @ k^T / sqrt(D))        # [q, k] rows sum to 1
    colsum[k] = sum_q P[q, k]
    values[k, :] = v[k, :] * colsum_h[k]
    out = values @ w_o

Sharding: 8 cores = 2 batches x 4 head-groups (4 heads each). Each core
computes qkv^T for its heads (all bf16), then per head-PAIR: scores for both
heads as concurrent K=64 row-tiled matmuls, exp on ACT (no accumulator reads
- row sums come from a DVE tensor_reduce over the exp tile), and the colsum
matvec for both heads col-tiled into one shared PSUM region (partitions 0-63
= head A, 64-127 = head B; DVE memset + start=False accumulation so no
bank-wide has_written clears corrupt the co-resident head). v^T is scaled
in one shot, then out = values @ w_o_slice streams out as bf16 partials
summed on the host.
"""

import sys

sys.path.insert(0, "/opt/trn_rl_repo")

import numpy as np

B, S, HIDDEN = 2, 2048, 1024
NH, HD = 16, 64
HPC = 4            # heads per core
FPC = 3 * HPC * HD # qkv feature columns per core (768)
N_CORES = 8
P = 128
QC = S // P        # 128-row q chunks (16)

_CACHE = {}


def _build():
    if "nc" in _CACHE:
        return _CACHE["nc"]

    import concourse.bass as bass
    import concourse.mybir as mybir
    import concourse.tile as tile
    from concourse import bacc

    F32 = mybir.dt.float32
    BF16 = mybir.dt.bfloat16
    EXP = mybir.ActivationFunctionType.Exp
    ADD = mybir.AluOpType.add
    MULT = mybir.AluOpType.mult
    AX_X = mybir.AxisListType.X

    nc = bacc.Bacc()
    xb_d = nc.declare_dram_parameter("xb", [HIDDEN, S], BF16, isOutput=False)
    wb_d = nc.declare_dram_parameter("wb", [HIDDEN, FPC], BF16, isOutput=False)
    wo_d = nc.declare_dram_parameter("wo", [2 * P, HIDDEN], BF16, isOutput=False)
    out_d = nc.declare_dram_parameter("out", [S, HIDDEN], BF16, isOutput=True)

    with tile.TileContext(nc) as tc:
        with tc.tile_pool(name="qkvt", bufs=1) as qkvt_pool, \
             tc.tile_pool(name="wsb", bufs=1) as w_pool:
            # qkvT tiles: 0,1 = Q pairs, 2,3 = K pairs, 4,5 = V pairs
            # (pair p tile: partitions 0-63 = head 2p, 64-127 = head 2p+1)
            qkvt = [qkvt_pool.tile([P, S], BF16, name=f"qkvt{mc}")
                    for mc in range(6)]
            wb = [w_pool.tile([P, FPC], BF16, name=f"wb{kc}") for kc in range(8)]
            wo = [w_pool.tile([P, HIDDEN], BF16, name=f"wo{kc}") for kc in range(2)]
            for kc in range(2):
                nc.sync.dma_start(out=wo[kc], in_=wo_d[kc * P:(kc + 1) * P, :])

            # ---- phase 1: qkv^T = (x @ w_qkv)^T, all bf16 ----
            with tc.tile_pool(name="xb", bufs=1) as x_pool:
                # HAM warmup: garbage matmuls hold the PE clock at 2.4 GHz
                # while the input DMAs stream in (values never read).
                with tc.tile_pool(name="psw", bufs=1, space="PSUM") as psw_pool:
                    wu = x_pool.tile([P, P], BF16, name="wu")
                    nc.vector.memset(wu, 0.0)
                    wu_ps = psw_pool.tile([P, P], F32, name="wups")
                    for _ in range(96):
                        nc.tensor.matmul(wu_ps, wu, wu, start=True, stop=True)

                xb = [x_pool.tile([P, S], BF16, name=f"xb{kc}") for kc in range(8)]
                for kc in range(8):
                    nc.sync.dma_start(out=wb[kc], in_=wb_d[kc * P:(kc + 1) * P, :])
                    nc.sync.dma_start(out=xb[kc], in_=xb_d[kc * P:(kc + 1) * P, :])
                # kc-outer over mc-PAIRS: the first pair's matmuls track the
                # x DMA chunk-by-chunk instead of waiting for all of x
                psq_ctx = tc.tile_pool(name="psq", bufs=4, space="PSUM")
                psq_pool = psq_ctx.__enter__()
                for mcp in range(3):
                    pst = [psq_pool.tile([P, 1024], F32, name="psq")
                           for _ in range(4)]
                    for kc in range(8):
                        for mi in range(2):
                            mc = 2 * mcp + mi
                            for nt in range(4):
                                nc.tensor.matmul(
                                    pst[2 * mi + nt // 2][
                                        :, (nt % 2) * 512:(nt % 2 + 1) * 512],
                                    wb[kc][:, mc * P:(mc + 1) * P],
                                    xb[kc][:, nt * 512:(nt + 1) * 512],
                                    start=(kc == 0), stop=(kc == 7))
                    for mi in range(2):
                        for h2 in range(2):
                            nc.vector.tensor_copy(
                                out=qkvt[2 * mcp + mi][
                                    :, h2 * 1024:(h2 + 1) * 1024],
                                in_=pst[2 * mi + h2])
                psq_ctx.__exit__(None, None, None)

            # ---- phase 2: per head-pair scores -> exp -> colsum -> v scale.
            # Unit order per q-chunk: A0 (ACT exp + fused row-sum accumulator),
            # A1, B0, B1 (plain ACT exp). Remaining row-sum pieces are DVE
            # reduces placed so each becomes ready just after its producer
            # ACTIVATE. Matvec for chunk qc-1 is emitted split (A after
            # A-scores, B after B-scores).
            with tc.tile_pool(name="esb", bufs=4) as e_pool, \
                 tc.tile_pool(name="zsb", bufs=8) as z_pool, \
                 tc.tile_pool(name="ps_s", bufs=2, space="PSUM") as ps_s_pool, \
                 tc.tile_pool(name="ps_c", bufs=1, space="PSUM") as ps_c_pool:
                for pr in range(2):
                    qt, kt, vt = qkvt[pr], qkvt[2 + pr], qkvt[4 + pr]
                    ps_c = ps_c_pool.tile([P, S], F32, name="psc")
                    nc.vector.memset(ps_c, 0.0)

                    st = {}

                    def emit_scores_h(ps0, ps1, h, qc):
                        bp = h * 64
                        for kh, ps in ((0, ps0), (1, ps1)):
                            for n2 in range(2):
                                c0 = kh * 1024 + n2 * 512
                                nc.tensor.matmul(
                                    ps[:, n2 * 512:(n2 + 1) * 512],
                                    qt[bp:bp + 64, qc * P:(qc + 1) * P],
                                    kt[bp:bp + 64, c0:c0 + 512],
                                    start=True, stop=True)

                    def emit_matvec_h(qc_p, h):
                        p = st[qc_p]
                        last = qc_p == QC - 1
                        wr = p["wrA"] if h == 0 else p["wrB"]
                        e = p["eA"] if h == 0 else p["eB"]
                        for j in range(4):
                            nc.tensor.matmul(
                                ps_c[h * 64:(h + 1) * 64,
                                     j * 512:(j + 1) * 512],
                                wr, e[:, j * 512:(j + 1) * 512],
                                start=False, stop=last, skip_group_check=True)

                    for qc in range(QC + 1):
                        if qc < QC:
                            eA = e_pool.tile([P, S], BF16, name="eA")
                            eB = e_pool.tile([P, S], BF16, name="eB")
                            st[qc] = dict(eA=eA, eB=eB)
                            # head A
                            psA0 = ps_s_pool.tile([P, 1024], F32, name="pss")
                            psA1 = ps_s_pool.tile([P, 1024], F32, name="pss")
                            emit_scores_h(psA0, psA1, 0, qc)
                            if qc >= 1:
                                emit_matvec_h(qc - 1, 0)
                            rA0 = z_pool.tile([P, 1], F32, name="rA0")
                            nc.scalar.activation(eA[:, 0:1024], psA0, EXP,
                                                 scale=0.125, accum_out=rA0)
                            nc.scalar.activation(eA[:, 1024:2048], psA1, EXP,
                                                 scale=0.125)
                            rA1 = z_pool.tile([P, 1], F32, name="rA1")
                            nc.vector.tensor_reduce(rA1, eA[:, 1024:2048],
                                                    AX_X, ADD)
                            zA = z_pool.tile([P, 1], F32, name="zA")
                            nc.vector.tensor_tensor(zA, rA0, rA1, ADD)
                            nc.vector.reciprocal(zA, zA)
                            wrA = z_pool.tile([P, 64], BF16, name="wrA")
                            nc.vector.tensor_copy(
                                out=wrA, in_=zA.to_broadcast([P, 64]))
                            st[qc]["wrA"] = wrA
                            # head B
                            psB0 = ps_s_pool.tile([P, 1024], F32, name="pss")
                            psB1 = ps_s_pool.tile([P, 1024], F32, name="pss")
                            emit_scores_h(psB0, psB1, 1, qc)
                            if qc >= 1:
                                emit_matvec_h(qc - 1, 1)
                            nc.scalar.activation(eB[:, 0:1024], psB0, EXP,
                                                 scale=0.125)
                            nc.scalar.activation(eB[:, 1024:2048], psB1, EXP,
                                                 scale=0.125)
                            rB0 = z_pool.tile([P, 1], F32, name="rB0")
                            nc.vector.tensor_reduce(rB0, eB[:, 0:1024],
                                                    AX_X, ADD)
                            rB1 = z_pool.tile([P, 1], F32, name="rB1")
                            nc.vector.tensor_reduce(rB1, eB[:, 1024:2048],
                                                    AX_X, ADD)
                            zB = z_pool.tile([P, 1], F32, name="zB")
                            nc.vector.tensor_tensor(zB, rB0, rB1, ADD)
                            nc.vector.reciprocal(zB, zB)
                            wrB = z_pool.tile([P, 64], BF16, name="wrB")
                            nc.vector.tensor_copy(
                                out=wrB, in_=zB.to_broadcast([P, 64]))
                            st[qc]["wrB"] = wrB
                        else:
                            emit_matvec_h(QC - 1, 0)
                            emit_matvec_h(QC - 1, 1)
                    # v^T *= colsum for both heads of the pair
                    nc.vector.tensor_tensor(vt, vt, ps_c, MULT)

            # ---- phase 4: out_partial = values @ w_o_slice  [s, hidden] ----
            with tc.tile_pool(name="osb", bufs=3) as o_pool, \
                 tc.tile_pool(name="ps_o", bufs=2, space="PSUM") as ps_o_pool:
                brg = ps_o_pool.tile([P, P], F32, name="brg")
                for _ in range(24):
                    nc.tensor.matmul(brg, wo[0][:, 0:P], wo[0][:, 0:P],
                                     start=True, stop=True)
                for sc in range(QC):
                    ps_o = ps_o_pool.tile([P, HIDDEN], F32, name="pso")
                    for nh in range(2):
                        for kc in range(2):
                            nc.tensor.matmul(
                                ps_o[:, nh * 512:(nh + 1) * 512],
                                qkvt[4 + kc][:, sc * P:(sc + 1) * P],
                                wo[kc][:, nh * 512:(nh + 1) * 512],
                                start=(kc == 0), stop=(kc == 1))
                    o_sb = o_pool.tile([P, HIDDEN], BF16, name="osb")
                    nc.vector.tensor_copy(out=o_sb, in_=ps_o)
                    nc.sync.dma_start(out=out_d[sc * P:(sc + 1) * P, :], in_=o_sb)

    nc.compile()
    _CACHE["nc"] = nc
    return nc


def kernel(x: np.ndarray, w_qkv: np.ndarray, w_o: np.ndarray) -> np.ndarray:
    import ml_dtypes
    from concourse.bass_utils import run_bass_kernel_spmd

    nc = _build()

    xT = [np.ascontiguousarray(x[b].T).astype(ml_dtypes.bfloat16)
          for b in range(B)]
    in_maps = []
    for c in range(N_CORES):
        b, g = divmod(c, HPC)
        wb = np.concatenate(
            [w_qkv[:, t * NH * HD + 256 * g: t * NH * HD + 256 * g + 256]
             for t in range(3)], axis=1).astype(ml_dtypes.bfloat16)
        wo_s = w_o[256 * g:256 * g + 256, :].astype(ml_dtypes.bfloat16)
        in_maps.append({"xb": xT[b], "wb": wb, "wo": wo_s})

    res = run_bass_kernel_spmd(nc, in_maps, list(range(N_CORES)),
                               **_CACHE.get("run_kwargs", {}))
    _CACHE["last_result"] = res

    out = np.zeros((B, S, HIDDEN), np.float32)
    for c in range(N_CORES):
        out[c // HPC] += res.results[c]["out"].astype(np.float32)
    return out


# revision 15
# speedup vs baseline: 1.2175x; 1.2175x over previous
"""Trainium2 Bass kernel for nn_BaselineAttention_36172214567310.

Reference computation (note the einsum 'bhqk,bhkd->bhkd' bug: the "attention
output" is v scaled by the column-sums of the softmax matrix):

    qkv = x @ w_qkv                       # [b, s, 3*H*D]
    q, k, v = split(qkv)                  # per head
    P = softmax(q @ k^T / sqrt(D))        # [q, k] rows sum to 1
    colsum[k] = sum_q P[q, k]
    values[k, :] = v[k, :] * colsum_h[k]
    out = values @ w_o

Sharding: 8 cores = 2 batches x 4 head-groups (4 heads each). Each core
computes qkv^T for its heads (all bf16), then per head-PAIR: scores for both
heads as concurrent K=64 row-tiled matmuls, exp on ACT (no accumulator reads
- row sums come from a DVE tensor_reduce over the exp tile), and the colsum
matvec for both heads col-tiled into one shared PSUM region (partitions 0-63
= head A, 64-127 = head B; DVE memset + start=False accumulation so no
bank-wide has_written clears corrupt the co-resident head). v^T is scaled
in one shot, then out = values @ w_o_slice streams out as bf16 partials
summed on the host.
"""

import sys

sys.path.insert(0, "/opt/trn_rl_repo")

import numpy as np

B, S, HIDDEN = 2, 2048, 1024
NH, HD = 16, 64
HPC = 4            # heads per core
FPC = 3 * HPC * HD # qkv feature columns per core (768)
N_CORES = 8
P = 128
QC = S // P        # 128-row q chunks (16)

_CACHE = {}


def _build():
    if "nc" in _CACHE:
        return _CACHE["nc"]

    import concourse.bass as bass
    import concourse.mybir as mybir
    import concourse.tile as tile
    from concourse import bacc

    F32 = mybir.dt.float32
    BF16 = mybir.dt.bfloat16
    EXP = mybir.ActivationFunctionType.Exp
    ADD = mybir.AluOpType.add
    MULT = mybir.AluOpType.mult
    AX_X = mybir.AxisListType.X

    nc = bacc.Bacc()
    xb_d = nc.declare_dram_parameter("xb", [HIDDEN, S], BF16, isOutput=False)
    wb_d = nc.declare_dram_parameter("wb", [HIDDEN, FPC], BF16, isOutput=False)
    wo_d = nc.declare_dram_parameter("wo", [2 * P, HIDDEN], BF16, isOutput=False)
    out_d = nc.declare_dram_parameter("out", [S, HIDDEN], BF16, isOutput=True)

    with tile.TileContext(nc) as tc:
        with tc.tile_pool(name="qkvt", bufs=1) as qkvt_pool, \
             tc.tile_pool(name="wsb", bufs=1) as w_pool:
            # qkvT tiles: 0,1 = Q pairs, 2,3 = K pairs, 4,5 = V pairs
            # (pair p tile: partitions 0-63 = head 2p, 64-127 = head 2p+1)
            qkvt = [qkvt_pool.tile([P, S], BF16, name=f"qkvt{mc}")
                    for mc in range(6)]
            wb = [w_pool.tile([P, FPC], BF16, name=f"wb{kc}") for kc in range(8)]
            wo = [w_pool.tile([P, HIDDEN], BF16, name=f"wo{kc}") for kc in range(2)]
            for kc in range(2):
                nc.sync.dma_start(out=wo[kc], in_=wo_d[kc * P:(kc + 1) * P, :])

            # ---- phase 1: qkv^T = (x @ w_qkv)^T, all bf16 ----
            with tc.tile_pool(name="xb", bufs=1) as x_pool:
                # HAM warmup: garbage matmuls hold the PE clock at 2.4 GHz
                # while the input DMAs stream in (values never read).
                with tc.tile_pool(name="psw", bufs=1, space="PSUM") as psw_pool:
                    wu = x_pool.tile([P, P], BF16, name="wu")
                    nc.vector.memset(wu, 0.0)
                    wu_ps = psw_pool.tile([P, P], F32, name="wups")
                    for _ in range(96):
                        nc.tensor.matmul(wu_ps, wu, wu, start=True, stop=True)

                xb = [x_pool.tile([P, S], BF16, name=f"xb{kc}") for kc in range(8)]
                for kc in range(8):
                    nc.sync.dma_start(out=wb[kc], in_=wb_d[kc * P:(kc + 1) * P, :])
                    nc.sync.dma_start(out=xb[kc], in_=xb_d[kc * P:(kc + 1) * P, :])
                psq_ctx = tc.tile_pool(name="psq", bufs=4, space="PSUM")
                psq_pool = psq_ctx.__enter__()
                for mc in range(6):
                    pst = [psq_pool.tile([P, 1024], F32, name="psq")
                           for _ in range(2)]
                    for kc in range(8):
                        for nt in range(4):
                            nc.tensor.matmul(
                                pst[nt // 2][:, (nt % 2) * 512:(nt % 2 + 1) * 512],
                                wb[kc][:, mc * P:(mc + 1) * P],
                                xb[kc][:, nt * 512:(nt + 1) * 512],
                                start=(kc == 0), stop=(kc == 7))
                    for h2 in range(2):
                        nc.vector.tensor_copy(
                            out=qkvt[mc][:, h2 * 1024:(h2 + 1) * 1024],
                            in_=pst[h2])
                psq_ctx.__exit__(None, None, None)

            # ---- phase 2: per head-pair scores -> exp -> colsum -> v scale.
            # Unit order per q-chunk: A0 (ACT exp + fused row-sum accumulator),
            # A1, B0, B1 (plain ACT exp). Remaining row-sum pieces are DVE
            # reduces placed so each becomes ready just after its producer
            # ACTIVATE. Matvec for chunk qc-1 is emitted split (A after
            # A-scores, B after B-scores).
            with tc.tile_pool(name="esb", bufs=4) as e_pool, \
                 tc.tile_pool(name="zsb", bufs=8) as z_pool, \
                 tc.tile_pool(name="ps_s", bufs=2, space="PSUM") as ps_s_pool, \
                 tc.tile_pool(name="ps_c", bufs=1, space="PSUM") as ps_c_pool:
                for pr in range(2):
                    qt, kt, vt = qkvt[pr], qkvt[2 + pr], qkvt[4 + pr]
                    ps_c = ps_c_pool.tile([P, S], F32, name="psc")
                    nc.vector.memset(ps_c, 0.0)

                    st = {}

                    def emit_scores_h(ps0, ps1, h, qc):
                        bp = h * 64
                        for kh, ps in ((0, ps0), (1, ps1)):
                            for n2 in range(2):
                                c0 = kh * 1024 + n2 * 512
                                nc.tensor.matmul(
                                    ps[:, n2 * 512:(n2 + 1) * 512],
                                    qt[bp:bp + 64, qc * P:(qc + 1) * P],
                                    kt[bp:bp + 64, c0:c0 + 512],
                                    start=True, stop=True)

                    def emit_matvec_h(qc_p, h):
                        p = st[qc_p]
                        last = qc_p == QC - 1
                        wr = p["wrA"] if h == 0 else p["wrB"]
                        e = p["eA"] if h == 0 else p["eB"]
                        for j in range(4):
                            nc.tensor.matmul(
                                ps_c[h * 64:(h + 1) * 64,
                                     j * 512:(j + 1) * 512],
                                wr, e[:, j * 512:(j + 1) * 512],
                                start=False, stop=last, skip_group_check=True)

                    for qc in range(QC + 1):
                        if qc < QC:
                            eA = e_pool.tile([P, S], BF16, name="eA")
                            eB = e_pool.tile([P, S], BF16, name="eB")
                            st[qc] = dict(eA=eA, eB=eB)
                            # head A
                            psA0 = ps_s_pool.tile([P, 1024], F32, name="pss")
                            psA1 = ps_s_pool.tile([P, 1024], F32, name="pss")
                            emit_scores_h(psA0, psA1, 0, qc)
                            if qc >= 1:
                                emit_matvec_h(qc - 1, 0)
                            rA0 = z_pool.tile([P, 1], F32, name="rA0")
                            nc.scalar.activation(eA[:, 0:1024], psA0, EXP,
                                                 scale=0.125, accum_out=rA0)
                            nc.scalar.activation(eA[:, 1024:2048], psA1, EXP,
                                                 scale=0.125)
                            rA1 = z_pool.tile([P, 1], F32, name="rA1")
                            nc.vector.tensor_reduce(rA1, eA[:, 1024:2048],
                                                    AX_X, ADD)
                            zA = z_pool.tile([P, 1], F32, name="zA")
                            nc.vector.tensor_tensor(zA, rA0, rA1, ADD)
                            nc.vector.reciprocal(zA, zA)
                            wrA = z_pool.tile([P, 64], BF16, name="wrA")
                            nc.vector.tensor_copy(
                                out=wrA, in_=zA.to_broadcast([P, 64]))
                            st[qc]["wrA"] = wrA
                            # head B
                            psB0 = ps_s_pool.tile([P, 1024], F32, name="pss")
                            psB1 = ps_s_pool.tile([P, 1024], F32, name="pss")
                            emit_scores_h(psB0, psB1, 1, qc)
                            if qc >= 1:
                                emit_matvec_h(qc - 1, 1)
                            nc.scalar.activation(eB[:, 0:1024], psB0, EXP,
                                                 scale=0.125)
                            nc.scalar.activation(eB[:, 1024:2048], psB1, EXP,
                                                 scale=0.125)
                            rB0 = z_pool.tile([P, 1], F32, name="rB0")
                            nc.vector.tensor_reduce(rB0, eB[:, 0:1024],
                                                    AX_X, ADD)
                            rB1 = z_pool.tile([P, 1], F32, name="rB1")
                            nc.vector.tensor_reduce(rB1, eB[:, 1024:2048],
                                                    AX_X, ADD)
                            zB = z_pool.tile([P, 1], F32, name="zB")
                            nc.vector.tensor_tensor(zB, rB0, rB1, ADD)
                            nc.vector.reciprocal(zB, zB)
                            wrB = z_pool.tile([P, 64], BF16, name="wrB")
                            nc.vector.tensor_copy(
                                out=wrB, in_=zB.to_broadcast([P, 64]))
                            st[qc]["wrB"] = wrB
                        else:
                            emit_matvec_h(QC - 1, 0)
                            emit_matvec_h(QC - 1, 1)
                    # v^T *= colsum for both heads of the pair
                    nc.vector.tensor_tensor(vt, vt, ps_c, MULT)

            # ---- phase 4: out_partial = values @ w_o_slice  [s, hidden] ----
            with tc.tile_pool(name="osb", bufs=3) as o_pool, \
                 tc.tile_pool(name="ps_o", bufs=2, space="PSUM") as ps_o_pool:
                brg = ps_o_pool.tile([P, P], F32, name="brg")
                for _ in range(24):
                    nc.tensor.matmul(brg, wo[0][:, 0:P], wo[0][:, 0:P],
                                     start=True, stop=True)
                for sc in range(QC):
                    ps_o = ps_o_pool.tile([P, HIDDEN], F32, name="pso")
                    for nh in range(2):
                        for kc in range(2):
                            nc.tensor.matmul(
                                ps_o[:, nh * 512:(nh + 1) * 512],
                                qkvt[4 + kc][:, sc * P:(sc + 1) * P],
                                wo[kc][:, nh * 512:(nh + 1) * 512],
                                start=(kc == 0), stop=(kc == 1))
                    o_sb = o_pool.tile([P, HIDDEN], BF16, name="osb")
                    nc.vector.tensor_copy(out=o_sb, in_=ps_o)
                    nc.sync.dma_start(out=out_d[sc * P:(sc + 1) * P, :], in_=o_sb)

    nc.compile()
    _CACHE["nc"] = nc
    return nc


def kernel(x: np.ndarray, w_qkv: np.ndarray, w_o: np.ndarray) -> np.ndarray:
    import ml_dtypes
    from concourse.bass_utils import run_bass_kernel_spmd

    nc = _build()

    xT = [np.ascontiguousarray(x[b].T).astype(ml_dtypes.bfloat16)
          for b in range(B)]
    in_maps = []
    for c in range(N_CORES):
        b, g = divmod(c, HPC)
        wb = np.concatenate(
            [w_qkv[:, t * NH * HD + 256 * g: t * NH * HD + 256 * g + 256]
             for t in range(3)], axis=1).astype(ml_dtypes.bfloat16)
        wo_s = w_o[256 * g:256 * g + 256, :].astype(ml_dtypes.bfloat16)
        in_maps.append({"xb": xT[b], "wb": wb, "wo": wo_s})

    res = run_bass_kernel_spmd(nc, in_maps, list(range(N_CORES)),
                               **_CACHE.get("run_kwargs", {}))
    _CACHE["last_result"] = res

    out = np.zeros((B, S, HIDDEN), np.float32)
    for c in range(N_CORES):
        out[c // HPC] += res.results[c]["out"].astype(np.float32)
    return out
